# revision 7
# baseline (speedup 1.0000x reference)
"""Chamfer loss kernel for Trainium2 (8 NeuronCores).

Problem: B=8 batches of point clouds pred/gt, each (3, 4096) f32.
loss = sum_b sum_j min_i d(pred_i, gt_j)/denom + sum_b sum_i min_j d(pred_i, gt_j)/denom
with d = Euclidean distance, denom = B * num_points.

Strategy:
 - Data-parallel: one batch per core (8 cores).
 - min commutes with sqrt(max(.,0)) => running min over squared distances,
   sqrt only the final 4096+4096 values per batch.
 - d2[i,j] = pn2[i] + gn2[j] - 2<p_i, g_j> computed entirely on the PE via an
   augmented matmul.  fp32 matmul runs at 1/4 rate on TRN2, so inputs are
   split into bf16 hi+lo parts (error ~1e-4 absolute on d2): K=13 rows
   cover hi*hi, hi*lo, lo*hi cross terms plus the two norm rows (hi+lo).
 - Flash-style min over gt-blocks: PE writes d2 tiles to PSUM; ScalarE copies
   half of each group to SBUF; VectorE tensor_tensor_scan(min, min) folds one
   PSUM tile + one SBUF tile per op (2 elements/cycle/partition on DVE).
 - Two passes: pass A (pred on partitions -> z2), pass B (gt on partitions -> z1).
 - Epilogue: relu, sqrt (ScalarE), row-sum -> [128, 2] per core; host sums.

This walrus build encodes at most ONE sync-wait per instruction; the
_split_waits pass hoists extra waits onto single-wait ENGINE_NOP carriers
(keeping a same-engine wait, if any, on the original instruction).
"""

import numpy as np

B = 8
D = 3
N = 4096
P = 128  # partitions (pred/gt chunk size)
NCHUNK = N // P  # 32 chunks of 128 points on partitions
FD = 512  # matmul free dim (one PSUM bank of fp32)
HC = 1024  # tile group: 2 matmuls -> one [128, 1024] PSUM tile (2 banks)
HC2 = 2048  # unit: 4 matmuls -> one [128, 2048] PSUM tile (4 banks)
K = 13  # augmented contraction rows
BIG = 3.0e38

_CACHE = {}

_ENGINE_SEM_PREFIX = {
    "EngineType.PE": "PE_",
    "EngineType.DVE": "DVE_",
    "EngineType.Activation": "Activation_",
    "EngineType.Pool": "Pool_",
    "EngineType.SP": "SP_",
}


def _split_waits(nc):
    """Walrus here encodes at most one sync-wait per instruction: hoist extra
    waits onto single-wait ENGINE_NOP carriers inserted just before, keeping a
    same-engine wait (cheapest to satisfy) on the original instruction."""
    import concourse.mybir as mybir

    def make_nop(engine):
        nop = mybir.InstNoOp(
            name=nc.get_next_instruction_name(), ins=[], outs=[], bass_nofuse=True
        )
        nop.engine = engine
        return nop

    total = 0
    for blk in nc.m.functions[0].blocks:
        insts = list(blk.instructions)
        newlist = []
        changed = False
        for inst in insts:
            si = getattr(inst, "sync_info", None)
            if si is not None and len(si.on_wait) > 1:
                waits = list(si.on_wait)
                pref = _ENGINE_SEM_PREFIX.get(str(inst.engine))
                keep_i = len(waits) - 1
                if pref is not None:
                    for i, w in enumerate(waits):
                        if w.ant_name and w.ant_name.startswith(pref):
                            keep_i = i
                            break
                keep = waits[keep_i]
                for i, w in enumerate(waits):
                    if i == keep_i:
                        continue
                    nop = make_nop(inst.engine)
                    nop.sync_info = mybir.SyncInfo(on_wait=[w], on_update=[])
                    newlist.append(nop)
                    total += 1
                inst.sync_info = mybir.SyncInfo(
                    on_wait=[keep], on_update=list(si.on_update)
                )
                changed = True
            newlist.append(inst)
        if changed:
            blk.instructions = newlist
    return total


def _sink_scan_waits(nc):
    """DVE scans wait on both the ps_d matmuls (PE sem) and the Act copy.
    _split_waits would hoist the extra wait onto a DVE NOP, costing ~90ns of
    DVE sequencer time per scan (~11us total).  Instead sink the Act wait
    onto the nearest preceding PE matmul with a free wait slot: PE has ~50%
    slack, and the move is deadlock-free because the ps_d matmuls come after
    the ps_a matmuls the copy depends on."""
    import concourse.mybir as mybir

    moved = 0
    for blk in nc.m.functions[0].blocks:
        insts = list(blk.instructions)
        for idx, inst in enumerate(insts):
            if not isinstance(inst, mybir.InstTensorScalarPtr):
                continue
            if str(inst.engine) != "EngineType.DVE":
                continue
            si = getattr(inst, "sync_info", None)
            if si is None or len(si.on_wait) < 2:
                continue
            act_i = None
            for i, w in enumerate(si.on_wait):
                if w.ant_name and w.ant_name.startswith("Activation_"):
                    act_i = i
                    break
            if act_i is None:
                continue
            # nearest preceding PE matmul with no wait yet
            target = None
            for j in range(idx - 1, max(idx - 8, -1), -1):
                pj = insts[j]
                if isinstance(pj, mybir.InstMatmult) and str(pj.engine) == (
                    "EngineType.PE"
                ):
                    sj = getattr(pj, "sync_info", None)
                    if sj is None or len(sj.on_wait) == 0:
                        target = pj
                        break
            if target is None:
                continue
            w = si.on_wait[act_i]
            rest = [x for i, x in enumerate(si.on_wait) if i != act_i]
            sj = getattr(target, "sync_info", None)
            target.sync_info = mybir.SyncInfo(
                on_wait=[w],
                on_update=list(sj.on_update) if sj is not None else [],
            )
            inst.sync_info = mybir.SyncInfo(
                on_wait=rest, on_update=list(si.on_update)
            )
            moved += 1
    return moved


def _build_bass(repeat=1):
    import concourse.bass as bass
    import concourse.mybir as mybir
    import concourse.tile as tile

    f32 = mybir.dt.float32
    bf16 = mybir.dt.bfloat16
    nc = bass.Bass(trn_type="TRN2")

    # packed [lhsA | rhsA | lhsB | rhsB] along the free axis
    inp = nc.dram_tensor("inp", [K, 4 * N], bf16, kind="ExternalInput")
    out = nc.dram_tensor("out", [P, 2], f32, kind="ExternalOutput")

    with tile.TileContext(nc) as tc:
        with (
            tc.tile_pool(name="inp", bufs=1) as inpool,
            tc.tile_pool(name="psum", bufs=2, space="PSUM") as psum_pool,
            tc.tile_pool(name="cp", bufs=3) as cp_pool,
            tc.tile_pool(name="acc", bufs=1) as acc_pool,
        ):
            inp_t = inpool.tile([K, 4 * N], bf16, tag="inp")
            # split load ordered by first use: chunk 0 needs only the first
            # 512 cols of lhsA but ALL of rhsA, so a small lhsA head-slice
            # goes first, then rhsA, then the lhsA tail and pass-B operands.
            spans = [
                (0, 512),          # lhsA head (chunks 0-3 weights)
                (N, 2 * N),        # rhsA (full, needed by every chunk)
                (512, N),          # lhsA tail
                (2 * N, 3 * N),    # lhsB
                (3 * N, 4 * N),    # rhsB
            ]
            for lo, hi in spans:
                nc.sync.dma_start(inp_t[:, lo:hi], inp[:, lo:hi])
            lhsA_t = inp_t[:, 0 * N : 1 * N]
            rhsA_t = inp_t[:, 1 * N : 2 * N]
            lhsB_t = inp_t[:, 2 * N : 3 * N]
            rhsB_t = inp_t[:, 3 * N : 4 * N]

            out_t = acc_pool.tile([P, 2], f32, tag="out")

            for _rep in range(repeat):
              for pidx, (lhs_t, rhs_t) in enumerate(
                [(lhsA_t, rhsA_t), (lhsB_t, rhsB_t)]
              ):
                acc = acc_pool.tile([P, 2 * NCHUNK], f32, tag=f"acc{pidx}")
                for c in range(NCHUNK):
                    lw = lhs_t[:, c * P : (c + 1) * P]  # [K, 128] stationary
                    for h in range(N // (2 * HC)):  # 2 groups of 2048 gt-points
                        # two PSUM tiles, each with exactly one reader engine
                        ps_d = psum_pool.tile([P, HC], f32, tag="ps_d")
                        ps_a = psum_pool.tile([P, HC], f32, tag="ps_a")
                        j0 = h * 2 * HC
                        for q in range(HC // FD):
                            j1 = j0 + HC
                            nc.tensor.matmul(
                                ps_a[:, q * FD : (q + 1) * FD],
                                lw,
                                rhs_t[:, j1 + q * FD : j1 + (q + 1) * FD],
                                start=True,
                                stop=True,
                            )
                        for q in range(HC // FD):
                            nc.tensor.matmul(
                                ps_d[:, q * FD : (q + 1) * FD],
                                lw,
                                rhs_t[:, j0 + q * FD : j0 + (q + 1) * FD],
                                start=True,
                                stop=True,
                            )
                        # ScalarE drains its PSUM tile to SBUF (bf16: halves
                        # SBUF traffic; min result unaffected beyond ~0.4%)
                        cp = cp_pool.tile([P, HC], bf16, tag="cp")
                        nc.scalar.copy(cp[:], ps_a[:])
                        # VectorE: running min across (psum tile, copy tile);
                        # stride-0 broadcast out => last write = block min
                        dst = acc[:, 2 * c + h : 2 * c + h + 1]
                        nc.vector.tensor_tensor_scan(
                            dst.broadcast_to((P, HC)),
                            ps_d[:],
                            cp[:],
                            initial=BIG,
                            op0=mybir.AluOpType.min,
                            op1=mybir.AluOpType.min,
                        )
                # pair-min -> relu -> sqrt -> row-sum
                acc_m = acc_pool.tile([P, NCHUNK], f32, tag=f"accm{pidx}")
                nc.vector.tensor_reduce(
                    acc_m[:],
                    acc[:].rearrange("p (c h) -> p c h", h=2),
                    axis=mybir.AxisListType.X,
                    op=mybir.AluOpType.min,
                )
                acc_r = acc_pool.tile([P, NCHUNK], f32, tag=f"accr{pidx}")
                nc.vector.tensor_scalar_max(acc_r[:], acc_m[:], 0.0)
                acc_s = acc_pool.tile([P, NCHUNK], f32, tag=f"accs{pidx}")
                nc.scalar.sqrt(acc_s[:], acc_r[:])
                nc.vector.reduce_sum(
                    out_t[:, pidx : pidx + 1], acc_s[:], axis=mybir.AxisListType.X
                )

            nc.sync.dma_start(out[:], out_t[:])

    _sink_scan_waits(nc)
    _split_waits(nc)
    return nc


def _hi_lo(x64):
    """x (fp64) -> (hi, lo) bf16 parts with hi + lo ~= x to ~2^-17 relative."""
    import ml_dtypes

    hi = x64.astype(ml_dtypes.bfloat16)
    lo = (x64 - hi.astype(np.float64)).astype(ml_dtypes.bfloat16)
    return hi, lo


def _aug_pair(a64, an2_64, b64, bn2_64):
    """lhsT/rhs augmented [K, N] bf16 pair so that (lhsT.T @ rhs)[i, j] ~=
    an2[i] + bn2[j] - 2 <a_i, b_j>."""
    import ml_dtypes

    a_hi, a_lo = _hi_lo(a64)
    b_hi, b_lo = _hi_lo(b64)
    an2_hi, an2_lo = _hi_lo(an2_64)
    bn2_hi, bn2_lo = _hi_lo(bn2_64)
    ones = np.ones((1, N), ml_dtypes.bfloat16)
    m2a_hi = (-2.0 * a_hi.astype(np.float64)).astype(ml_dtypes.bfloat16)  # exact
    m2a_lo = (-2.0 * a_lo.astype(np.float64)).astype(ml_dtypes.bfloat16)  # exact
    lhsT = np.concatenate(
        [m2a_hi, m2a_hi, m2a_lo, ones, ones, an2_hi[None, :], an2_lo[None, :]],
        axis=0,
    )
    rhs = np.concatenate(
        [b_hi, b_lo, b_hi, bn2_hi[None, :], bn2_lo[None, :], ones, ones],
        axis=0,
    )
    return lhsT, rhs


def _prep_core_inputs(p, g):
    """p, g: (3, N) f32 for one batch -> packed augmented matmul operands."""
    p64 = p.astype(np.float64)
    g64 = g.astype(np.float64)
    pn2 = (p64 * p64).sum(axis=0)
    gn2 = (g64 * g64).sum(axis=0)
    lhsA, rhsA = _aug_pair(p64, pn2, g64, gn2)
    lhsB, rhsB = _aug_pair(g64, gn2, p64, pn2)
    packed = np.concatenate([lhsA, rhsA, lhsB, rhsB], axis=1)
    assert packed.shape == (K, 4 * N)
    return {"inp": np.ascontiguousarray(packed)}


def kernel(predict_pc, gt_pc, num_points, _trace=False):
    from concourse.bass_utils import run_bass_kernel_spmd

    pred = np.ascontiguousarray(np.asarray(predict_pc), dtype=np.float32)
    gt = np.ascontiguousarray(np.asarray(gt_pc), dtype=np.float32)
    batch = gt.shape[0]
    assert pred.shape == (B, D, N) and gt.shape == (B, D, N)

    if "nc" not in _CACHE:
        _CACHE["nc"] = _build_bass()
    nc = _CACHE["nc"]

    in_maps = [_prep_core_inputs(pred[b], gt[b]) for b in range(B)]
    res = run_bass_kernel_spmd(
        nc, in_maps, core_ids=list(range(B)), trace=_trace
    )
    kernel.last_results = res

    total = 0.0
    for b in range(B):
        o = res.results[b]["out"].astype(np.float64)
        total += o.sum()  # col 0 = z2 partial sums, col 1 = z1 partial sums
    denom = float(batch) * float(num_points)
    return np.asarray(np.float64(total) / denom, dtype=np.float32)



# revision 8
# speedup vs baseline: 1.0009x; 1.0009x over previous
"""Chamfer loss kernel for Trainium2 (8 NeuronCores).

Problem: B=8 batches of point clouds pred/gt, each (3, 4096) f32.
loss = sum_b sum_j min_i d(pred_i, gt_j)/denom + sum_b sum_i min_j d(pred_i, gt_j)/denom
with d = Euclidean distance, denom = B * num_points.

Strategy:
 - Data-parallel: one batch per core (8 cores).
 - min commutes with sqrt(max(.,0)) => running min over squared distances,
   sqrt only the final 4096+4096 values per batch.
 - d2[i,j] = pn2[i] + gn2[j] - 2<p_i, g_j> computed entirely on the PE via an
   augmented matmul.  fp32 matmul runs at 1/4 rate on TRN2, so inputs are
   split into bf16 hi+lo parts (error ~1e-4 absolute on d2): K=13 rows
   cover hi*hi, hi*lo, lo*hi cross terms plus the two norm rows (hi+lo).
 - Flash-style min over gt-blocks: PE writes d2 tiles to PSUM; ScalarE copies
   half of each group to SBUF; VectorE tensor_tensor_scan(min, min) folds one
   PSUM tile + one SBUF tile per op (2 elements/cycle/partition on DVE).
 - Two passes: pass A (pred on partitions -> z2), pass B (gt on partitions -> z1).
 - Epilogue: relu, sqrt (ScalarE), row-sum -> [128, 2] per core; host sums.

This walrus build encodes at most ONE sync-wait per instruction; the
_split_waits pass hoists extra waits onto single-wait ENGINE_NOP carriers
(keeping a same-engine wait, if any, on the original instruction).
"""

import numpy as np

B = 8
D = 3
N = 4096
P = 128  # partitions (pred/gt chunk size)
NCHUNK = N // P  # 32 chunks of 128 points on partitions
FD = 512  # matmul free dim (one PSUM bank of fp32)
HC = 1024  # tile group: 2 matmuls -> one [128, 1024] PSUM tile (2 banks)
HC2 = 2048  # unit: 4 matmuls -> one [128, 2048] PSUM tile (4 banks)
K = 13  # augmented contraction rows
BIG = 3.0e38

_CACHE = {}

_ENGINE_SEM_PREFIX = {
    "EngineType.PE": "PE_",
    "EngineType.DVE": "DVE_",
    "EngineType.Activation": "Activation_",
    "EngineType.Pool": "Pool_",
    "EngineType.SP": "SP_",
}


def _split_waits(nc):
    """Walrus here encodes at most one sync-wait per instruction: hoist extra
    waits onto single-wait ENGINE_NOP carriers inserted just before, keeping a
    same-engine wait (cheapest to satisfy) on the original instruction."""
    import concourse.mybir as mybir

    def make_nop(engine):
        nop = mybir.InstNoOp(
            name=nc.get_next_instruction_name(), ins=[], outs=[], bass_nofuse=True
        )
        nop.engine = engine
        return nop

    total = 0
    for blk in nc.m.functions[0].blocks:
        insts = list(blk.instructions)
        newlist = []
        changed = False
        for inst in insts:
            si = getattr(inst, "sync_info", None)
            if si is not None and len(si.on_wait) > 1:
                waits = list(si.on_wait)
                pref = _ENGINE_SEM_PREFIX.get(str(inst.engine))
                keep_i = len(waits) - 1
                if pref is not None:
                    for i, w in enumerate(waits):
                        if w.ant_name and w.ant_name.startswith(pref):
                            keep_i = i
                            break
                keep = waits[keep_i]
                for i, w in enumerate(waits):
                    if i == keep_i:
                        continue
                    nop = make_nop(inst.engine)
                    nop.sync_info = mybir.SyncInfo(on_wait=[w], on_update=[])
                    newlist.append(nop)
                    total += 1
                inst.sync_info = mybir.SyncInfo(
                    on_wait=[keep], on_update=list(si.on_update)
                )
                changed = True
            newlist.append(inst)
        if changed:
            blk.instructions = newlist
    return total


def _sink_scan_waits(nc):
    """DVE scans wait on both the ps_d matmuls (PE sem) and the Act copy.
    _split_waits would hoist the extra wait onto a DVE NOP, costing ~90ns of
    DVE sequencer time per scan (~11us total).  Instead sink the Act wait
    onto the nearest preceding PE matmul with a free wait slot: PE has ~50%
    slack, and the move is deadlock-free because the ps_d matmuls come after
    the ps_a matmuls the copy depends on."""
    import concourse.mybir as mybir

    moved = 0
    for blk in nc.m.functions[0].blocks:
        insts = list(blk.instructions)
        for idx, inst in enumerate(insts):
            if not isinstance(inst, mybir.InstTensorScalarPtr):
                continue
            if str(inst.engine) != "EngineType.DVE":
                continue
            si = getattr(inst, "sync_info", None)
            if si is None or len(si.on_wait) < 2:
                continue
            act_i = None
            for i, w in enumerate(si.on_wait):
                if w.ant_name and w.ant_name.startswith("Activation_"):
                    act_i = i
                    break
            if act_i is None:
                continue
            # nearest preceding PE matmul with no wait yet
            target = None
            for j in range(idx - 1, max(idx - 8, -1), -1):
                pj = insts[j]
                if isinstance(pj, mybir.InstMatmult) and str(pj.engine) == (
                    "EngineType.PE"
                ):
                    sj = getattr(pj, "sync_info", None)
                    if sj is None or len(sj.on_wait) == 0:
                        target = pj
                        break
            if target is None:
                continue
            w = si.on_wait[act_i]
            rest = [x for i, x in enumerate(si.on_wait) if i != act_i]
            sj = getattr(target, "sync_info", None)
            target.sync_info = mybir.SyncInfo(
                on_wait=[w],
                on_update=list(sj.on_update) if sj is not None else [],
            )
            inst.sync_info = mybir.SyncInfo(
                on_wait=rest, on_update=list(si.on_update)
            )
            moved += 1
    return moved


def _build_bass(repeat=1):
    import concourse.bass as bass
    import concourse.mybir as mybir
    import concourse.tile as tile

    f32 = mybir.dt.float32
    bf16 = mybir.dt.bfloat16
    nc = bass.Bass(trn_type="TRN2")

    # packed [lhsA | rhsA | lhsB | rhsB] along the free axis
    inp = nc.dram_tensor("inp", [K, 4 * N], bf16, kind="ExternalInput")
    out = nc.dram_tensor("out", [P, 2], f32, kind="ExternalOutput")

    with tile.TileContext(nc) as tc:
        with (
            tc.tile_pool(name="inp", bufs=1) as inpool,
            tc.tile_pool(name="psum", bufs=2, space="PSUM") as psum_pool,
            tc.tile_pool(name="cp", bufs=3) as cp_pool,
            tc.tile_pool(name="acc", bufs=1) as acc_pool,
        ):
            inp_t = inpool.tile([K, 4 * N], bf16, tag="inp")
            # split load ordered by first use: chunk 0 needs only the first
            # 512 cols of lhsA but ALL of rhsA, so a small lhsA head-slice
            # goes first, then rhsA, then the lhsA tail and pass-B operands.
            spans = [
                (0, 512),          # lhsA head (chunks 0-3 weights)
                (N, N + HC2),      # rhsA head (chunk 0 group h=0)
                (N + HC2, 2 * N),  # rhsA tail
                (512, N),          # lhsA tail
                (2 * N, 3 * N),    # lhsB
                (3 * N, 4 * N),    # rhsB
            ]
            for lo, hi in spans:
                nc.sync.dma_start(inp_t[:, lo:hi], inp[:, lo:hi])
            lhsA_t = inp_t[:, 0 * N : 1 * N]
            rhsA_t = inp_t[:, 1 * N : 2 * N]
            lhsB_t = inp_t[:, 2 * N : 3 * N]
            rhsB_t = inp_t[:, 3 * N : 4 * N]

            out_t = acc_pool.tile([P, 2], f32, tag="out")

            for _rep in range(repeat):
              for pidx, (lhs_t, rhs_t) in enumerate(
                [(lhsA_t, rhsA_t), (lhsB_t, rhsB_t)]
              ):
                acc = acc_pool.tile([P, 2 * NCHUNK], f32, tag=f"acc{pidx}")
                for c in range(NCHUNK):
                    lw = lhs_t[:, c * P : (c + 1) * P]  # [K, 128] stationary
                    for h in range(N // (2 * HC)):  # 2 groups of 2048 gt-points
                        # two PSUM tiles, each with exactly one reader engine
                        ps_d = psum_pool.tile([P, HC], f32, tag="ps_d")
                        ps_a = psum_pool.tile([P, HC], f32, tag="ps_a")
                        j0 = h * 2 * HC
                        for q in range(HC // FD):
                            j1 = j0 + HC
                            nc.tensor.matmul(
                                ps_a[:, q * FD : (q + 1) * FD],
                                lw,
                                rhs_t[:, j1 + q * FD : j1 + (q + 1) * FD],
                                start=True,
                                stop=True,
                            )
                        for q in range(HC // FD):
                            nc.tensor.matmul(
                                ps_d[:, q * FD : (q + 1) * FD],
                                lw,
                                rhs_t[:, j0 + q * FD : j0 + (q + 1) * FD],
                                start=True,
                                stop=True,
                            )
                        # ScalarE drains its PSUM tile to SBUF (bf16: halves
                        # SBUF traffic; min result unaffected beyond ~0.4%)
                        cp = cp_pool.tile([P, HC], bf16, tag="cp")
                        nc.scalar.copy(cp[:], ps_a[:])
                        # VectorE: running min across (psum tile, copy tile);
                        # stride-0 broadcast out => last write = block min
                        dst = acc[:, 2 * c + h : 2 * c + h + 1]
                        nc.vector.tensor_tensor_scan(
                            dst.broadcast_to((P, HC)),
                            ps_d[:],
                            cp[:],
                            initial=BIG,
                            op0=mybir.AluOpType.min,
                            op1=mybir.AluOpType.min,
                        )
                # pair-min -> relu -> sqrt -> row-sum
                acc_m = acc_pool.tile([P, NCHUNK], f32, tag=f"accm{pidx}")
                nc.vector.tensor_reduce(
                    acc_m[:],
                    acc[:].rearrange("p (c h) -> p c h", h=2),
                    axis=mybir.AxisListType.X,
                    op=mybir.AluOpType.min,
                )
                acc_r = acc_pool.tile([P, NCHUNK], f32, tag=f"accr{pidx}")
                nc.vector.tensor_scalar_max(acc_r[:], acc_m[:], 0.0)
                acc_s = acc_pool.tile([P, NCHUNK], f32, tag=f"accs{pidx}")
                nc.scalar.sqrt(acc_s[:], acc_r[:])
                nc.vector.reduce_sum(
                    out_t[:, pidx : pidx + 1], acc_s[:], axis=mybir.AxisListType.X
                )

            nc.sync.dma_start(out[:], out_t[:])

    _sink_scan_waits(nc)
    _split_waits(nc)
    return nc


def _hi_lo(x64):
    """x (fp64) -> (hi, lo) bf16 parts with hi + lo ~= x to ~2^-17 relative."""
    import ml_dtypes

    hi = x64.astype(ml_dtypes.bfloat16)
    lo = (x64 - hi.astype(np.float64)).astype(ml_dtypes.bfloat16)
    return hi, lo


def _aug_pair(a64, an2_64, b64, bn2_64):
    """lhsT/rhs augmented [K, N] bf16 pair so that (lhsT.T @ rhs)[i, j] ~=
    an2[i] + bn2[j] - 2 <a_i, b_j>."""
    import ml_dtypes

    a_hi, a_lo = _hi_lo(a64)
    b_hi, b_lo = _hi_lo(b64)
    an2_hi, an2_lo = _hi_lo(an2_64)
    bn2_hi, bn2_lo = _hi_lo(bn2_64)
    ones = np.ones((1, N), ml_dtypes.bfloat16)
    m2a_hi = (-2.0 * a_hi.astype(np.float64)).astype(ml_dtypes.bfloat16)  # exact
    m2a_lo = (-2.0 * a_lo.astype(np.float64)).astype(ml_dtypes.bfloat16)  # exact
    lhsT = np.concatenate(
        [m2a_hi, m2a_hi, m2a_lo, ones, ones, an2_hi[None, :], an2_lo[None, :]],
        axis=0,
    )
    rhs = np.concatenate(
        [b_hi, b_lo, b_hi, bn2_hi[None, :], bn2_lo[None, :], ones, ones],
        axis=0,
    )
    return lhsT, rhs


def _prep_core_inputs(p, g):
    """p, g: (3, N) f32 for one batch -> packed augmented matmul operands."""
    p64 = p.astype(np.float64)
    g64 = g.astype(np.float64)
    pn2 = (p64 * p64).sum(axis=0)
    gn2 = (g64 * g64).sum(axis=0)
    lhsA, rhsA = _aug_pair(p64, pn2, g64, gn2)
    lhsB, rhsB = _aug_pair(g64, gn2, p64, pn2)
    packed = np.concatenate([lhsA, rhsA, lhsB, rhsB], axis=1)
    assert packed.shape == (K, 4 * N)
    return {"inp": np.ascontiguousarray(packed)}


def kernel(predict_pc, gt_pc, num_points, _trace=False):
    from concourse.bass_utils import run_bass_kernel_spmd

    pred = np.ascontiguousarray(np.asarray(predict_pc), dtype=np.float32)
    gt = np.ascontiguousarray(np.asarray(gt_pc), dtype=np.float32)
    batch = gt.shape[0]
    assert pred.shape == (B, D, N) and gt.shape == (B, D, N)

    if "nc" not in _CACHE:
        _CACHE["nc"] = _build_bass()
    nc = _CACHE["nc"]

    in_maps = [_prep_core_inputs(pred[b], gt[b]) for b in range(B)]
    res = run_bass_kernel_spmd(
        nc, in_maps, core_ids=list(range(B)), trace=_trace
    )
    kernel.last_results = res

    total = 0.0
    for b in range(B):
        o = res.results[b]["out"].astype(np.float64)
        total += o.sum()  # col 0 = z2 partial sums, col 1 = z1 partial sums
    denom = float(batch) * float(num_points)
    return np.asarray(np.float64(total) / denom, dtype=np.float32)



# revision 9
# speedup vs baseline: 1.0025x; 1.0016x over previous
"""Chamfer loss kernel for Trainium2 (8 NeuronCores).

Problem: B=8 batches of point clouds pred/gt, each (3, 4096) f32.
loss = sum_b sum_j min_i d(pred_i, gt_j)/denom + sum_b sum_i min_j d(pred_i, gt_j)/denom
with d = Euclidean distance, denom = B * num_points.

Strategy:
 - Data-parallel: one batch per core (8 cores).
 - min commutes with sqrt(max(.,0)) => running min over squared distances,
   sqrt only the final 4096+4096 values per batch.
 - d2[i,j] = pn2[i] + gn2[j] - 2<p_i, g_j> computed entirely on the PE via an
   augmented matmul.  fp32 matmul runs at 1/4 rate on TRN2, so inputs are
   split into bf16 hi+lo parts (error ~1e-4 absolute on d2): K=13 rows
   cover hi*hi, hi*lo, lo*hi cross terms plus the two norm rows (hi+lo).
 - Flash-style min over gt-blocks: PE writes d2 tiles to PSUM; ScalarE copies
   half of each group to SBUF; VectorE tensor_tensor_scan(min, min) folds one
   PSUM tile + one SBUF tile per op (2 elements/cycle/partition on DVE).
 - Two passes: pass A (pred on partitions -> z2), pass B (gt on partitions -> z1).
 - Epilogue: relu, sqrt (ScalarE), row-sum -> [128, 2] per core; host sums.

This walrus build encodes at most ONE sync-wait per instruction; the
_split_waits pass hoists extra waits onto single-wait ENGINE_NOP carriers
(keeping a same-engine wait, if any, on the original instruction).
"""

import numpy as np

B = 8
D = 3
N = 4096
P = 128  # partitions (pred/gt chunk size)
NCHUNK = N // P  # 32 chunks of 128 points on partitions
FD = 512  # matmul free dim (one PSUM bank of fp32)
HC = 1024  # tile group: 2 matmuls -> one [128, 1024] PSUM tile (2 banks)
HC2 = 2048  # unit: 4 matmuls -> one [128, 2048] PSUM tile (4 banks)
K = 13  # augmented contraction rows
BIG = 3.0e38

_CACHE = {}

_ENGINE_SEM_PREFIX = {
    "EngineType.PE": "PE_",
    "EngineType.DVE": "DVE_",
    "EngineType.Activation": "Activation_",
    "EngineType.Pool": "Pool_",
    "EngineType.SP": "SP_",
}


def _split_waits(nc):
    """Walrus here encodes at most one sync-wait per instruction: hoist extra
    waits onto single-wait ENGINE_NOP carriers inserted just before, keeping a
    same-engine wait (cheapest to satisfy) on the original instruction."""
    import concourse.mybir as mybir

    def make_nop(engine):
        nop = mybir.InstNoOp(
            name=nc.get_next_instruction_name(), ins=[], outs=[], bass_nofuse=True
        )
        nop.engine = engine
        return nop

    total = 0
    for blk in nc.m.functions[0].blocks:
        insts = list(blk.instructions)
        newlist = []
        changed = False
        for inst in insts:
            si = getattr(inst, "sync_info", None)
            if si is not None and len(si.on_wait) > 1:
                waits = list(si.on_wait)
                pref = _ENGINE_SEM_PREFIX.get(str(inst.engine))
                keep_i = len(waits) - 1
                if pref is not None:
                    for i, w in enumerate(waits):
                        if w.ant_name and w.ant_name.startswith(pref):
                            keep_i = i
                            break
                keep = waits[keep_i]
                for i, w in enumerate(waits):
                    if i == keep_i:
                        continue
                    nop = make_nop(inst.engine)
                    nop.sync_info = mybir.SyncInfo(on_wait=[w], on_update=[])
                    newlist.append(nop)
                    total += 1
                inst.sync_info = mybir.SyncInfo(
                    on_wait=[keep], on_update=list(si.on_update)
                )
                changed = True
            newlist.append(inst)
        if changed:
            blk.instructions = newlist
    return total


def _sink_scan_waits(nc):
    """DVE scans wait on both the ps_d matmuls (PE sem) and the Act copy.
    _split_waits would hoist the extra wait onto a DVE NOP, costing ~90ns of
    DVE sequencer time per scan (~11us total).  Instead sink the Act wait
    onto the nearest preceding PE matmul with a free wait slot: PE has ~50%
    slack, and the move is deadlock-free because the ps_d matmuls come after
    the ps_a matmuls the copy depends on."""
    import concourse.mybir as mybir

    moved = 0
    for blk in nc.m.functions[0].blocks:
        insts = list(blk.instructions)
        for idx, inst in enumerate(insts):
            if not isinstance(inst, mybir.InstTensorScalarPtr):
                continue
            if str(inst.engine) != "EngineType.DVE":
                continue
            si = getattr(inst, "sync_info", None)
            if si is None or len(si.on_wait) < 2:
                continue
            act_i = None
            for i, w in enumerate(si.on_wait):
                if w.ant_name and w.ant_name.startswith("Activation_"):
                    act_i = i
                    break
            if act_i is None:
                continue
            # nearest preceding PE matmul with no wait yet
            target = None
            for j in range(idx - 1, max(idx - 8, -1), -1):
                pj = insts[j]
                if isinstance(pj, mybir.InstMatmult) and str(pj.engine) == (
                    "EngineType.PE"
                ):
                    sj = getattr(pj, "sync_info", None)
                    if sj is None or len(sj.on_wait) == 0:
                        target = pj
                        break
            if target is None:
                continue
            w = si.on_wait[act_i]
            rest = [x for i, x in enumerate(si.on_wait) if i != act_i]
            sj = getattr(target, "sync_info", None)
            target.sync_info = mybir.SyncInfo(
                on_wait=[w],
                on_update=list(sj.on_update) if sj is not None else [],
            )
            inst.sync_info = mybir.SyncInfo(
                on_wait=rest, on_update=list(si.on_update)
            )
            moved += 1
    return moved


def _build_bass(repeat=1):
    import concourse.bass as bass
    import concourse.mybir as mybir
    import concourse.tile as tile

    f32 = mybir.dt.float32
    bf16 = mybir.dt.bfloat16
    nc = bass.Bass(trn_type="TRN2")

    # packed [lhsA | rhsA | lhsB | rhsB] along the free axis
    inp = nc.dram_tensor("inp", [K, 4 * N], bf16, kind="ExternalInput")
    out = nc.dram_tensor("out", [P, 2], f32, kind="ExternalOutput")

    with tile.TileContext(nc) as tc:
        with (
            tc.tile_pool(name="inp", bufs=1) as inpool,
            tc.tile_pool(name="psum", bufs=2, space="PSUM") as psum_pool,
            tc.tile_pool(name="cp", bufs=3) as cp_pool,
            tc.tile_pool(name="acc", bufs=1) as acc_pool,
        ):
            inp_t = inpool.tile([K, 4 * N], bf16, tag="inp")
            # split load ordered by first use: chunk 0 needs only the first
            # 512 cols of lhsA but ALL of rhsA, so a small lhsA head-slice
            # goes first, then rhsA, then the lhsA tail and pass-B operands.
            spans = [
                (0, 512),          # lhsA head (chunks 0-3 weights)
                (N, N + HC2),      # rhsA head (chunk 0 group h=0)
                (N + HC2, 2 * N),  # rhsA tail
                (512, N),          # lhsA tail
                (2 * N, 3 * N),    # lhsB
                (3 * N, 4 * N),    # rhsB
            ]
            for lo, hi in spans:
                nc.sync.dma_start(inp_t[:, lo:hi], inp[:, lo:hi])
            lhsA_t = inp_t[:, 0 * N : 1 * N]
            rhsA_t = inp_t[:, 1 * N : 2 * N]
            lhsB_t = inp_t[:, 2 * N : 3 * N]
            rhsB_t = inp_t[:, 3 * N : 4 * N]

            out_t = acc_pool.tile([P, 2], f32, tag="out")

            for _rep in range(repeat):
              for pidx, (lhs_t, rhs_t) in enumerate(
                [(lhsA_t, rhsA_t), (lhsB_t, rhsB_t)]
              ):
                acc = acc_pool.tile([P, 2 * NCHUNK], f32, tag=f"acc{pidx}")
                for c in range(NCHUNK):
                    lw = lhs_t[:, c * P : (c + 1) * P]  # [K, 128] stationary
                    for h in range(N // (2 * HC)):  # 2 groups of 2048 gt-points
                        # two PSUM tiles, each with exactly one reader engine
                        ps_d = psum_pool.tile([P, HC], f32, tag="ps_d")
                        ps_a = psum_pool.tile([P, HC], f32, tag="ps_a")
                        j0 = h * 2 * HC
                        for q in range(HC // FD):
                            j1 = j0 + HC
                            nc.tensor.matmul(
                                ps_a[:, q * FD : (q + 1) * FD],
                                lw,
                                rhs_t[:, j1 + q * FD : j1 + (q + 1) * FD],
                                start=True,
                                stop=True,
                            )
                        for q in range(HC // FD):
                            nc.tensor.matmul(
                                ps_d[:, q * FD : (q + 1) * FD],
                                lw,
                                rhs_t[:, j0 + q * FD : j0 + (q + 1) * FD],
                                start=True,
                                stop=True,
                            )
                        # ScalarE drains its PSUM tile to SBUF (bf16: halves
                        # SBUF traffic; min result unaffected beyond ~0.4%)
                        cp = cp_pool.tile([P, HC], bf16, tag="cp")
                        nc.scalar.copy(cp[:], ps_a[:])
                        # VectorE: running min across (psum tile, copy tile);
                        # stride-0 broadcast out => last write = block min
                        dst = acc[:, 2 * c + h : 2 * c + h + 1]
                        nc.vector.tensor_tensor_scan(
                            dst.broadcast_to((P, HC)),
                            ps_d[:],
                            cp[:],
                            initial=BIG,
                            op0=mybir.AluOpType.min,
                            op1=mybir.AluOpType.min,
                        )
                # pair-min -> relu -> sqrt -> row-sum
                acc_m = acc_pool.tile([P, NCHUNK], f32, tag=f"accm{pidx}")
                nc.vector.tensor_reduce(
                    acc_m[:],
                    acc[:].rearrange("p (c h) -> p c h", h=2),
                    axis=mybir.AxisListType.X,
                    op=mybir.AluOpType.min,
                )
                acc_r = acc_pool.tile([P, NCHUNK], f32, tag=f"accr{pidx}")
                nc.vector.tensor_scalar_max(acc_r[:], acc_m[:], 0.0)
                acc_s = acc_pool.tile([P, NCHUNK], f32, tag=f"accs{pidx}")
                nc.scalar.sqrt(acc_s[:], acc_r[:])
                nc.vector.reduce_sum(
                    out_t[:, pidx : pidx + 1], acc_s[:], axis=mybir.AxisListType.X
                )

            nc.sync.dma_start(out[:], out_t[:])

    _split_waits(nc)
    return nc


def _hi_lo(x64):
    """x (fp64) -> (hi, lo) bf16 parts with hi + lo ~= x to ~2^-17 relative."""
    import ml_dtypes

    hi = x64.astype(ml_dtypes.bfloat16)
    lo = (x64 - hi.astype(np.float64)).astype(ml_dtypes.bfloat16)
    return hi, lo


def _aug_pair(a64, an2_64, b64, bn2_64):
    """lhsT/rhs augmented [K, N] bf16 pair so that (lhsT.T @ rhs)[i, j] ~=
    an2[i] + bn2[j] - 2 <a_i, b_j>."""
    import ml_dtypes

    a_hi, a_lo = _hi_lo(a64)
    b_hi, b_lo = _hi_lo(b64)
    an2_hi, an2_lo = _hi_lo(an2_64)
    bn2_hi, bn2_lo = _hi_lo(bn2_64)
    ones = np.ones((1, N), ml_dtypes.bfloat16)
    m2a_hi = (-2.0 * a_hi.astype(np.float64)).astype(ml_dtypes.bfloat16)  # exact
    m2a_lo = (-2.0 * a_lo.astype(np.float64)).astype(ml_dtypes.bfloat16)  # exact
    lhsT = np.concatenate(
        [m2a_hi, m2a_hi, m2a_lo, ones, ones, an2_hi[None, :], an2_lo[None, :]],
        axis=0,
    )
    rhs = np.concatenate(
        [b_hi, b_lo, b_hi, bn2_hi[None, :], bn2_lo[None, :], ones, ones],
        axis=0,
    )
    return lhsT, rhs


def _prep_core_inputs(p, g):
    """p, g: (3, N) f32 for one batch -> packed augmented matmul operands."""
    p64 = p.astype(np.float64)
    g64 = g.astype(np.float64)
    pn2 = (p64 * p64).sum(axis=0)
    gn2 = (g64 * g64).sum(axis=0)
    lhsA, rhsA = _aug_pair(p64, pn2, g64, gn2)
    lhsB, rhsB = _aug_pair(g64, gn2, p64, pn2)
    packed = np.concatenate([lhsA, rhsA, lhsB, rhsB], axis=1)
    assert packed.shape == (K, 4 * N)
    return {"inp": np.ascontiguousarray(packed)}


def kernel(predict_pc, gt_pc, num_points, _trace=False):
    from concourse.bass_utils import run_bass_kernel_spmd

    pred = np.ascontiguousarray(np.asarray(predict_pc), dtype=np.float32)
    gt = np.ascontiguousarray(np.asarray(gt_pc), dtype=np.float32)
    batch = gt.shape[0]
    assert pred.shape == (B, D, N) and gt.shape == (B, D, N)

    if "nc" not in _CACHE:
        _CACHE["nc"] = _build_bass()
    nc = _CACHE["nc"]

    in_maps = [_prep_core_inputs(pred[b], gt[b]) for b in range(B)]
    res = run_bass_kernel_spmd(
        nc, in_maps, core_ids=list(range(B)), trace=_trace
    )
    kernel.last_results = res

    total = 0.0
    for b in range(B):
        o = res.results[b]["out"].astype(np.float64)
        total += o.sum()  # col 0 = z2 partial sums, col 1 = z1 partial sums
    denom = float(batch) * float(num_points)
    return np.asarray(np.float64(total) / denom, dtype=np.float32)

